# revision 1
# baseline (speedup 1.0000x reference)
"""Trainium2 Bass kernel for nn_LinearKAN (histogram_binning).

Math
----
reference computes, per (batch b, out o):

    out[b,o] = sum_i  PL_interp(x[b,i]; bp[o,i,:], val[o,i,:])

where bp is the SAME sorted uniform grid for every (o,i) (tiled
linspace).  With u = (x - bp0)/h in [0, S), the piecewise-linear
interpolant has an exact *clamp basis* expansion

    f(u) = val_0 + sum_{s=0..S-1} M_s * clamp(u - s, 0, 1)
    M_s  = val_{s+1} - val_s              (segment slopes)

so the layer is a bias plus S dense matmuls contracting over (s, i).

Device mapping (v2, single fp16 stream):
  - One-instruction clamp: g_s = min(max(u_q, a), a+1) with a = s%4 and
    u_q = u - 4*(s//4).  The block shift keeps every fp16 saturation an
    exact small integer, and the "- a" offset folds into the bias:
    bias_o = sum_i val0[o,i] - sum_s (s%4) * sum_i M_s[o,i].
  - u0 = (x - bp0)/h is precomputed on HOST in f64 and shipped as fp16
    (128 KB/core instead of 256 KB fp32 x); u_q tiles are exact fp16
    subtractions of 4q.  End-to-end rel err ~5.7e-3 (numpy-simulated),
    well under the 2e-2 gate.
  - M fp16 single stream (no hi/lo split): 40 K=128 matmuls per core,
    C traffic 1.31 MB/core.
  - bias is seeded INTO PSUM by two K=1 matmuls (bias_hi x ones,
    bias_lo*2048 x ones*2^-11) during the PE warmup window, so the tail
    is just one ACT copy PSUM->SBUF + DMA out (fp16).
  - shard: batch in 4 quarters (B_loc=256) x out-features in 2 halves
    (O_loc=128) over 8 cores; no cross-device reduction.

Scheduling notes (from trace iteration):
  - all bulk DMA rides the SP HWDGE ring in strict order (u0, then C
    chunks 6/10/12/12) -- per-partition rows >= 1.5KB, since the movers
    are descriptor-rate-bound (~116ns/packet/engine) below ~2KB rows;
    the ACT ring carries only the tiny bias row and the output.
  - g_0..g_3 are produced on DVE straight from u0 BEFORE the u_q chain
    so the first matmuls are gated only by the first C chunk.
  - never put tensor_scalar on Pool/gpsimd (software loop, ~7.5us/op).
  - the PE clock boost (HAM) arrives ~5-7us after the matmul stream
    starts and is granted in ~6.8us quanta; dummy warmup matmuls bridge
    the pre-stream gap so PE duty is continuous from ~1us in.
"""

import os
import numpy as np

import concourse.bass as bass
import concourse.mybir as mybir
import concourse.tile as tile
from concourse import bacc
from concourse.bass_utils import run_bass_kernel_spmd

# Problem shape (hardcoded per the task contract).
B, O, I, S = 1024, 256, 256, 20
N_CORES = 8
B_SPLIT, O_SPLIT = 4, 2
B_LOC, O_LOC = B // B_SPLIT, O // O_SPLIT  # 256, 128
KT = 2 * S          # 40 K-tiles of 128 over the (s, i) contraction
F32 = mybir.dt.float32
F16 = mybir.dt.float16
FW = 2 * B_LOC      # free width of u/g tiles: both i-halves side by side
QW = 4              # u-block width: u_q = u - 4q, q = 0..4

LO_ONES = 2.0 ** -11   # seed-matmul rhs for the bias lo part
LO_SCALE = 2048.0      # host pre-scale of bias lo (keeps fp16 normal)


def _envtuple(name, default):
    v = os.environ.get(name)
    if not v:
        return default
    return tuple(int(t) for t in v.split(",") if t != "")


# --- tunables (env-overridable for perf iteration) ---
N_WARMUP_MM = int(os.environ.get("KAN_WARMUP", "10"))  # PE clock-warmup mms
WARM_N = int(os.environ.get("KAN_WARM_N", "512"))     # warmup rhs width
CHUNK_KT = _envtuple("KAN_CHUNKS", (10, 10, 10, 10))  # C DMA chunk sizes
# Pool (gpsimd) tensor_scalar is a ~7.5us software loop on Q7 -- never
# put g ops there (measured).  ACT can do relu-only s (top segment) and
# the u_q Copy-with-bias tiles.
POOL_S = set(_envtuple("KAN_POOL_S", ()))             # g ops on Pool engine
ACT_S = set(_envtuple("KAN_ACT_S", (19,)))            # relu-only s on ACT
ACT_UQ = set(_envtuple("KAN_ACT_UQ", (3, 4)))         # u_q built on ACT


def _strip_init_boilerplate(nc) -> None:
    """Drop the Bass-init const-AP memsets + all-engine barrier (~1.5us of
    preamble).  All activation biases here are explicit APs or float biases
    on Copy, so the const-AP memsets and their barrier are dead weight."""
    blk = nc.m.functions[0].blocks[0]
    drop = (mybir.InstMemset, mybir.InstDrain, mybir.InstEventSemaphore)
    keep = [i for i in blk.instructions if not isinstance(i, drop)]
    del blk.instructions[:]
    for i in keep:
        blk.instructions.append(i)
    nc.const_aps.aps.clear()


def _build_nc() -> bass.Bass:
    """Build the (SPMD-identical) single-core Bass graph."""
    assert sum(CHUNK_KT) == KT, CHUNK_KT
    nc = bacc.Bacc("TRN2", target_bir_lowering=False, debug=False)
    _strip_init_boilerplate(nc)

    u0d = nc.declare_dram_parameter("u0", [128, FW], F16, isOutput=False)
    Cd = nc.declare_dram_parameter("C", [128, KT * 128], F16, isOutput=False)
    b2d = nc.declare_dram_parameter("b2", [1, 256], F16, isOutput=False)
    out = nc.declare_dram_parameter("out", [O_LOC, B_LOC], F16, isOutput=True)

    with tile.TileContext(nc) as tc:
        with (
            tc.tile_pool(name="u", bufs=6) as upool,
            tc.tile_pool(name="g", bufs=S) as gpool,
            tc.tile_pool(name="c", bufs=len(CHUNK_KT)) as cpool,
            tc.tile_pool(name="w", bufs=4) as wpool,
            tc.tile_pool(name="o", bufs=1) as opool,
            tc.tile_pool(name="ps", bufs=2, space="PSUM") as pspool,
        ):
            # --- Pool-engine memsets (no DMA dep).  Warmup operands
            # FIRST: they gate the PE duty ramp that opens the clock.
            if N_WARMUP_MM:
                # Full-array warmups (K=128): the HAM boost was only ever
                # observed at warmup_start+4.8us with >=4us of K=128
                # full-array duty (baseline recipe); thin K=1 warmups
                # leave the boost to arrive ~5us into the real stream.
                wa = wpool.tile([128, 128], F16, tag="warm_a")
                wb = wpool.tile([128, WARM_N], F16, tag="warm_b")
                nc.gpsimd.memset(wa[:], 0.0)
                nc.gpsimd.memset(wb[:], 0.0)
            ones_hi = wpool.tile([1, B_LOC], F16, tag="ones_hi")
            ones_lo = wpool.tile([1, B_LOC], F16, tag="ones_lo")
            nc.gpsimd.memset(ones_hi[:], 1.0)
            nc.gpsimd.memset(ones_lo[:], LO_ONES)
            actb = wpool.tile([128, len(ACT_S) or 1], F32, tag="actb")
            for k, s in enumerate(sorted(ACT_S)):
                nc.gpsimd.memset(actb[:, k:k + 1], -float(s))

            # --- PE HAM warmup: dummy matmuls on memset scratch so the
            # clock-gate opens (1.2 -> 2.4 GHz) before the real stream.
            if N_WARMUP_MM:
                ps_warm = pspool.tile([128, WARM_N], F32, tag="pw")
                for _ in range(N_WARMUP_MM):
                    nc.tensor.matmul(ps_warm[:], wa[:], wb[:],
                                     start=True, stop=True)

            # --- DMA in (sync HWDGE): u0 first (gates the whole
            # elementwise chain), tiny bias row next (gates the psum
            # seed), then C chunks smallest-first.
            # All bulk data rides ONE ring (Sync) in strict order --
            # u0 first, then C chunks small-to-large -- so mover
            # arbitration can't starve the transfers that gate compute.
            # The ACT ring only carries the tiny bias row (and later the
            # output), giving issue-parallelism without data competition.
            b2 = wpool.tile([1, 256], F16, tag="b2")
            nc.scalar.dma_start(b2[:], b2d[:])
            u0 = upool.tile([128, FW], F16, tag="u0")
            nc.sync.dma_start(u0[:], u0d[:])
            ckt = {}
            kt0 = 0
            for ci, nkt in enumerate(CHUNK_KT):
                t = cpool.tile([128, nkt * 128], F16, tag=f"c{ci}")
                nc.sync.dma_start(t[:], Cd[:, kt0 * 128:(kt0 + nkt) * 128])
                for k in range(nkt):
                    ckt[kt0 + k] = t[:, k * 128:(k + 1) * 128]
                kt0 += nkt

            # --- PSUM bias seed: ps = bias_hi + bias_lo*2048 * 2^-11 ---
            ps = pspool.tile([O_LOC, B_LOC], F32, tag="ps")
            nc.tensor.matmul(ps[:], b2[:, 0:O_LOC], ones_hi[:],
                             start=True, stop=False, skip_group_check=True)
            nc.tensor.matmul(ps[:], b2[:, O_LOC:2 * O_LOC], ones_lo[:],
                             start=False, stop=False, skip_group_check=True)

            # --- g_s = min(max(u_q, a), a+1), a = s%4; one op each.
            # The q=0 block (s=0..3) comes FIRST on DVE -- it needs only
            # u0 and gates the first matmuls; the u_q tiles (u_q = u0-4q,
            # fp16-exact for every value that matters) are built next,
            # early blocks on DVE, late ones on the otherwise-idle ACT
            # engine (Copy with float bias).
            g = [None] * S
            for s in range(QW):
                gs = gpool.tile([128, FW], F16, tag="g")
                nc.vector.tensor_scalar(
                    gs[:], u0[:], float(s), float(s) + 1.0,
                    mybir.AluOpType.max, mybir.AluOpType.min)
                g[s] = gs
            uq = {0: u0}
            for q in range(1, (S + QW - 1) // QW):
                t = upool.tile([128, FW], F16, tag="uq")
                if q in ACT_UQ:
                    nc.scalar.activation(
                        t[:], u0[:], mybir.ActivationFunctionType.Copy,
                        bias=-float(QW * q), scale=1.0)
                else:
                    nc.vector.tensor_scalar(
                        t[:], u0[:], float(QW * q), None,
                        mybir.AluOpType.subtract)
                uq[q] = t

            for s in range(QW, S):
                gs = gpool.tile([128, FW], F16, tag="g")
                q, a = s // QW, float(s % QW)
                if s in ACT_S:
                    # u < S strictly, so clamp(u-s,0,1) = relu(u-s) for
                    # the top segment; ACT engine is otherwise idle.
                    k = sorted(ACT_S).index(s)
                    nc.scalar.activation(
                        gs[:], u0[:], mybir.ActivationFunctionType.Relu,
                        bias=actb[:, k:k + 1], scale=1.0)
                elif s in POOL_S:
                    nc.gpsimd.tensor_scalar(
                        gs[:], uq[q][:], a, a + 1.0,
                        mybir.AluOpType.max, mybir.AluOpType.min)
                else:
                    nc.vector.tensor_scalar(
                        gs[:], uq[q][:], a, a + 1.0,
                        mybir.AluOpType.max, mybir.AluOpType.min)
                g[s] = gs

            # --- 40 accumulating matmuls over kt = (s, ih) ---
            for kt in range(KT):
                s, ih = kt // 2, kt % 2
                rhs = g[s][:, ih * B_LOC:(ih + 1) * B_LOC]
                nc.tensor.matmul(ps[:], ckt[kt], rhs,
                                 start=False, stop=(kt == KT - 1),
                                 skip_group_check=True)

            # --- tail: one ACT copy PSUM -> SBUF, then DMA out ---
            out_sb = opool.tile([O_LOC, B_LOC], F16, tag="osb")
            nc.scalar.copy(out_sb[:], ps[:])
            nc.scalar.dma_start(out[:], out_sb[:])
    nc.compile()
    return nc


_NC_CACHE: dict = {}


def _get_nc() -> bass.Bass:
    if "nc" not in _NC_CACHE:
        _NC_CACHE["nc"] = _build_nc()
    return _NC_CACHE["nc"]


def prepare(x: np.ndarray, breakpoints: np.ndarray, values: np.ndarray):
    """Host prep: build the Bass graph (cached) + per-core input maps."""
    x = np.asarray(x)
    values = np.asarray(values)

    # Grid affine params from the (shared) breakpoint row.
    bpr = np.asarray(breakpoints)[0, 0].astype(np.float64)
    h = (bpr[-1] - bpr[0]) / S
    scale = 1.0 / h
    ubias = -float(bpr[0]) / h

    # u in [0, S) computed on host in f64, shipped fp16.
    u = (x.astype(np.float64) * scale + ubias)
    u16 = u.astype(np.float16)

    # Clamp-basis slopes (fp16) and the folded bias (f64 -> hi/lo fp16).
    Vf = values.astype(np.float64)          # [O, I, S+1]
    M = (Vf[:, :, 1:] - Vf[:, :, :-1]).transpose(2, 0, 1)  # [S, O, I]
    M16 = M.astype(np.float16)
    # ACT-assigned s produce the UNshifted clamp (relu), so their fold
    # offset is 0, not s%4.
    amod = np.array([0.0 if s in ACT_S else float(s % QW) for s in range(S)])
    bias_o = Vf[:, :, 0].sum(axis=1) - np.einsum(
        "s,soi->o", amod, M, optimize=True)   # [O] f64
    bh = bias_o.astype(np.float16)
    bl = ((bias_o - bh.astype(np.float64)) * LO_SCALE).astype(np.float16)

    # Per-core layouts.
    M16_r = M16.reshape(S, O_SPLIT, O_LOC, 2, 128)  # [s, oh, o, ih, j]
    ur = u16.reshape(B_SPLIT, B_LOC, 2, 128)        # [bq, b, ih, j]

    in_maps = []
    for c in range(N_CORES):
        bq, oh = c % B_SPLIT, c // B_SPLIT
        # ur[bq] axes (b, ih, j) -> (j, ih, b) -> [128, FW]
        u0_c = np.ascontiguousarray(
            ur[bq].transpose(2, 1, 0)).reshape(128, FW)
        # [s, o, ih, j] -> (j, s, ih, o): columns kt*128 + o, kt = 2s+ih
        C_c = np.ascontiguousarray(
            M16_r[:, oh].transpose(3, 0, 2, 1)).reshape(128, KT * 128)
        b2_c = np.ascontiguousarray(np.concatenate(
            [bh[oh * O_LOC:(oh + 1) * O_LOC],
             bl[oh * O_LOC:(oh + 1) * O_LOC]]).reshape(1, 256))
        in_maps.append({"u0": u0_c, "C": C_c, "b2": b2_c})

    nc = _get_nc()
    return nc, in_maps


def kernel(x: np.ndarray, breakpoints: np.ndarray, values: np.ndarray,
           **_extra) -> np.ndarray:
    nc, in_maps = prepare(x, breakpoints, values)
    res = run_bass_kernel_spmd(nc, in_maps, list(range(N_CORES)))

    outf = np.empty((B, O), np.float32)
    for c in range(N_CORES):
        bq, oh = c % B_SPLIT, c // B_SPLIT
        outf[bq * B_LOC:(bq + 1) * B_LOC, oh * O_LOC:(oh + 1) * O_LOC] = \
            res.results[c]["out"].T.astype(np.float32)
    return outf


if __name__ == "__main__":
    rng = np.random.default_rng(0)
    x = rng.uniform(-1, 1, (B, I)).astype(np.float32)
    bp = np.tile(np.linspace(-1, 1, S + 1, dtype=np.float32), (O, I, 1))
    v = (rng.standard_normal((O, I, S + 1)) * 0.1).astype(np.float32)
    out = kernel(x, bp, v)
    print("kernel ran, out:", out.shape, out.dtype, float(out.std()))



# revision 6
# speedup vs baseline: 1.0076x; 1.0076x over previous
"""Trainium2 Bass kernel for nn_LinearKAN (histogram_binning).

Math
----
reference computes, per (batch b, out o):

    out[b,o] = sum_i  PL_interp(x[b,i]; bp[o,i,:], val[o,i,:])

where bp is the SAME sorted uniform grid for every (o,i) (tiled
linspace).  With u = (x - bp0)/h in [0, S), the piecewise-linear
interpolant has an exact *clamp basis* expansion

    f(u) = val_0 + sum_{s=0..S-1} M_s * clamp(u - s, 0, 1)
    M_s  = val_{s+1} - val_s              (segment slopes)

so the layer is a bias plus S dense matmuls contracting over (s, i).

Device mapping (v3):
  - UNSHIFTED clamp basis: gt_s = min(max(u, s), s+1).  For u (already
    fp16) in (s, s+1) the clamp is a passthrough, and the integer
    saturations are exact in fp16, so the unshifted basis adds NO
    rounding over the shifted one -- PROVIDED the host bias fold is
    computed against the fp16-QUANTIZED device weights (the old fold
    against exact f64 slopes is what made large shifts lossy).
  - u-substitution: sum_s clamp(u-s,0,1) = u identically on [0,S), so
    the s=0 basis function is replaced by u0 itself as a matmul rhs
    (weights D_u = fp16(M_0); other weights become M_s - M_0).  One
    fewer DVE op, no extra tiles.
  - top segment s=19 on the ACT engine as relu(u-19) (u < 20 strictly).
  - 18 interior clamps on DVE, one dual-ALU tensor_scalar each.
  - bias is seeded into PSUM by ONE K=2 matmul (rows: bias_hi,
    bias_lo*2048; rhs rows: ones, ones*2^-11) during the PE warmup
    window; tail is a split PSUM->SBUF copy (DVE half + ACT half) +
    one DMA out (fp16).
  - shard: batch quarters (B_loc=256) x out-feature halves (O_loc=128)
    over 8 cores; no cross-device reduction.

Scheduling notes (from trace iteration):
  - exec_time is measured from the FIRST kernel instruction to the
    absolute end of the NEFF teardown (the ~250-instruction per-engine
    semaphore-reset storm, ~8-9us, is framework-fixed and fully
    counted).  Only the active window [first memset -> out-DMA done]
    is controllable.
  - the 6.5us framework preamble before the first kernel instruction
    is NOT counted.
  - bulk DMA: u0 first on the Sync HWDGE ring (it gates everything),
    the first C chunk rides the ACT ring in parallel, remaining C
    chunks follow u0 on Sync.  Rows >= 2.5KB, movers are
    descriptor-rate-bound (~116ns/packet/engine) below ~2KB rows.
  - PE warmup matmuls (memset scratch) keep full-array duty from
    ~7.0us so the HAM clock boost (observed onset ~13.5us, granted in
    ~6.8us quanta) arrives as early as possible; warmups are N=256 so
    the PE frees within ~1 op of data arrival.  Thin K=1 warmups do
    NOT earn the boost (measured in a previous session).
  - never put tensor_scalar on Pool/gpsimd (software loop, ~7.5us/op).
"""

import os
import numpy as np

import concourse.bass as bass
import concourse.mybir as mybir
import concourse.tile as tile
from concourse import bacc
from concourse.bass_utils import run_bass_kernel_spmd

# Problem shape (hardcoded per the task contract).
B, O, I, S = 1024, 256, 256, 20
N_CORES = 8
B_SPLIT, O_SPLIT = 4, 2
B_LOC, O_LOC = B // B_SPLIT, O // O_SPLIT  # 256, 128
KT = 2 * S          # 40 K-tiles of 128 over the (s, i) contraction
F32 = mybir.dt.float32
F16 = mybir.dt.float16
FW = 2 * B_LOC      # free width of u/g tiles: both i-halves side by side

# Bias is seeded hi/lo: row0 = fp16(bias), row1 = fp16(bias - row0).
# The residual is <= 0.125 (half an fp16 ulp at |bias|~300), comfortably
# normal in fp16, so both rows multiply a single all-ones rhs.


def _envtuple(name, default):
    v = os.environ.get(name)
    if not v:
        return default
    return tuple(int(t) for t in v.split(",") if t != "")


# --- tunables (env-overridable for perf iteration) ---
N_WARMUP_MM = int(os.environ.get("KAN_WARMUP", "10"))  # PE clock-warmup mms
WARM_N = int(os.environ.get("KAN_WARM_N", "256"))      # warmup rhs width
ACT_KT = int(os.environ.get("KAN_ACT_KT", "10"))       # C kt on the ACT ring
CHUNK_KT = _envtuple("KAN_CHUNKS", (10, 10, 10))       # C kt chunks, Sync ring
TAIL_SPLIT = int(os.environ.get("KAN_TAIL_SPLIT", "1"))  # split psum copy


def _strip_init_boilerplate(nc) -> None:
    """Drop the Bass-init const-AP memsets + all-engine barrier (~1.5us of
    preamble).  All activation biases here are explicit APs or float biases
    on Copy, so the const-AP memsets and their barrier are dead weight."""
    blk = nc.m.functions[0].blocks[0]
    drop = (mybir.InstMemset, mybir.InstDrain, mybir.InstEventSemaphore)
    keep = [i for i in blk.instructions if not isinstance(i, drop)]
    del blk.instructions[:]
    for i in keep:
        blk.instructions.append(i)
    nc.const_aps.aps.clear()


def _build_nc() -> bass.Bass:
    """Build the (SPMD-identical) single-core Bass graph."""
    assert ACT_KT + sum(CHUNK_KT) == KT, (ACT_KT, CHUNK_KT)
    nc = bacc.Bacc("TRN2", target_bir_lowering=False, debug=False)
    _strip_init_boilerplate(nc)

    u0d = nc.declare_dram_parameter("u0", [128, FW], F16, isOutput=False)
    Cd = nc.declare_dram_parameter("C", [128, KT * 128], F16, isOutput=False)
    b2d = nc.declare_dram_parameter("b2", [2, O_LOC], F16, isOutput=False)
    out = nc.declare_dram_parameter("out", [O_LOC, B_LOC], F16, isOutput=True)

    with tile.TileContext(nc) as tc:
        with (
            tc.tile_pool(name="u", bufs=1) as upool,
            tc.tile_pool(name="g", bufs=S) as gpool,
            tc.tile_pool(name="c", bufs=1 + len(CHUNK_KT)) as cpool,
            tc.tile_pool(name="w", bufs=4) as wpool,
            tc.tile_pool(name="o", bufs=1) as opool,
            tc.tile_pool(name="ps", bufs=2, space="PSUM") as pspool,
        ):
            # --- Pool-engine memsets (no DMA dep).  Warmup operand FIRST:
            # it gates the PE duty ramp that opens the clock boost.
            if N_WARMUP_MM:
                wa = wpool.tile([128, max(128, WARM_N)], F16, tag="warm_a")
                nc.gpsimd.memset(wa[:], 0.0)
            ones2 = wpool.tile([2, B_LOC], F16, tag="ones2")
            nc.gpsimd.memset(ones2[:], 1.0)
            actb = wpool.tile([128, 1], F32, tag="actb")
            nc.gpsimd.memset(actb[:], -float(S - 1))

            # --- PE HAM warmup: dummy matmuls on memset scratch keep
            # full-array duty up while waiting for data.
            if N_WARMUP_MM:
                ps_warm = pspool.tile([128, WARM_N], F32, tag="pw")
                for _ in range(N_WARMUP_MM):
                    nc.tensor.matmul(ps_warm[:], wa[:, 0:128], wa[:, 0:WARM_N],
                                     start=True, stop=True)

            # --- DMA in.  ACT ring: tiny bias row, then the first C chunk
            # (in parallel with u0).  Sync ring: u0 FIRST (it gates the
            # whole elementwise chain), then the remaining C chunks.
            b2 = wpool.tile([2, O_LOC], F16, tag="b2")
            nc.scalar.dma_start(b2[:], b2d[:])
            ckt = {}
            if ACT_KT:
                tA = cpool.tile([128, ACT_KT * 128], F16, tag="cA")
                nc.scalar.dma_start(tA[:], Cd[:, 0:ACT_KT * 128])
                for k in range(ACT_KT):
                    ckt[k] = tA[:, k * 128:(k + 1) * 128]
            u0 = upool.tile([128, FW], F16, tag="u0")
            nc.sync.dma_start(u0[:], u0d[:])
            kt0 = ACT_KT
            for ci, nkt in enumerate(CHUNK_KT):
                t = cpool.tile([128, nkt * 128], F16, tag=f"c{ci}")
                nc.sync.dma_start(t[:], Cd[:, kt0 * 128:(kt0 + nkt) * 128])
                for k in range(nkt):
                    ckt[kt0 + k] = t[:, k * 128:(k + 1) * 128]
                kt0 += nkt

            # --- PSUM bias seed: one K=2 matmul, ps = bias_hi + bias_lo.
            ps = pspool.tile([O_LOC, B_LOC], F32, tag="ps")
            nc.tensor.matmul(ps[:], b2[:], ones2[:],
                             start=True, stop=False, skip_group_check=True)

            # --- basis tiles: s=0 is u0 itself (u-substitution); interior
            # s=1..18 are single dual-ALU clamps on DVE; s=19 is relu on ACT.
            g = [None] * S
            g[0] = u0
            for s in range(1, S - 1):
                gs = gpool.tile([128, FW], F16, tag="g")
                nc.vector.tensor_scalar(
                    gs[:], u0[:], float(s), float(s) + 1.0,
                    mybir.AluOpType.max, mybir.AluOpType.min)
                g[s] = gs
            gt = gpool.tile([128, FW], F16, tag="g")
            nc.scalar.activation(
                gt[:], u0[:], mybir.ActivationFunctionType.Relu,
                bias=actb[:, 0:1], scale=1.0)
            g[S - 1] = gt

            # --- 40 accumulating matmuls over kt = (s, ih) ---
            for kt in range(KT):
                s, ih = kt // 2, kt % 2
                rhs = g[s][:, ih * B_LOC:(ih + 1) * B_LOC]
                nc.tensor.matmul(ps[:], ckt[kt], rhs,
                                 start=False, stop=(kt == KT - 1),
                                 skip_group_check=True)

            # --- tail: split PSUM -> SBUF copy (DVE + ACT halves run in
            # parallel), then one DMA out on the Sync ring.
            out_sb = opool.tile([O_LOC, B_LOC], F16, tag="osb")
            if TAIL_SPLIT:
                h = B_LOC // 2
                nc.vector.tensor_scalar(
                    out_sb[:, 0:h], ps[:, 0:h], 0.0, None,
                    mybir.AluOpType.add)
                nc.scalar.copy(out_sb[:, h:B_LOC], ps[:, h:B_LOC])
            else:
                nc.scalar.copy(out_sb[:], ps[:])
            nc.sync.dma_start(out[:], out_sb[:])
    nc.compile()
    return nc


_NC_CACHE: dict = {}


def _get_nc() -> bass.Bass:
    if "nc" not in _NC_CACHE:
        _NC_CACHE["nc"] = _build_nc()
    return _NC_CACHE["nc"]


def prepare(x: np.ndarray, breakpoints: np.ndarray, values: np.ndarray):
    """Host prep: build the Bass graph (cached) + per-core input maps."""
    x = np.asarray(x)
    values = np.asarray(values)

    # Grid affine params from the (shared) breakpoint row.
    bpr = np.asarray(breakpoints)[0, 0].astype(np.float64)
    h = (bpr[-1] - bpr[0]) / S
    scale = 1.0 / h
    ubias = -float(bpr[0]) / h

    # u in [0, S) computed on host in f64, shipped fp16.
    u = (x.astype(np.float64) * scale + ubias)
    u16 = u.astype(np.float16)

    # Clamp-basis slopes.  Device weights (all fp16):
    #   kt(s=0) slots: D_u = fp16(M_0)          (rhs = u0 itself)
    #   kt(s>=1) slots: D_s = fp16(M_s - D_u)   (rhs = clamp / relu tiles)
    # Bias fold MUST use the quantized device weights: matching at u=0
    # (all clamps = s, relu = 0, u-term = 0) gives
    #   bias_o = sum_i val0 - sum_{s=1..18} s * sum_i D_s[o,i].
    Vf = values.astype(np.float64)          # [O, I, S+1]
    M = (Vf[:, :, 1:] - Vf[:, :, :-1]).transpose(2, 0, 1)  # [S, O, I] f64
    Du = M[0].astype(np.float16)            # [O, I]
    D16 = np.empty((S, O, I), np.float16)
    D16[0] = Du
    Duf = Du.astype(np.float64)
    for s in range(1, S):
        D16[s] = (M[s] - Duf).astype(np.float16)
    svec = np.arange(1, S - 1, dtype=np.float64)  # 1..18
    bias_o = Vf[:, :, 0].sum(axis=1) - np.einsum(
        "s,soi->o", svec, D16[1:S - 1].astype(np.float64))  # [O] f64
    bh = bias_o.astype(np.float16)
    bl = (bias_o - bh.astype(np.float64)).astype(np.float16)

    # Per-core layouts.
    D16_r = D16.reshape(S, O_SPLIT, O_LOC, 2, 128)  # [s, oh, o, ih, j]
    ur = u16.reshape(B_SPLIT, B_LOC, 2, 128)        # [bq, b, ih, j]

    in_maps = []
    for c in range(N_CORES):
        bq, oh = c % B_SPLIT, c // B_SPLIT
        # ur[bq] axes (b, ih, j) -> (j, ih, b) -> [128, FW]
        u0_c = np.ascontiguousarray(
            ur[bq].transpose(2, 1, 0)).reshape(128, FW)
        # [s, o, ih, j] -> (j, s, ih, o): columns kt*128 + o, kt = 2s+ih
        C_c = np.ascontiguousarray(
            D16_r[:, oh].transpose(3, 0, 2, 1)).reshape(128, KT * 128)
        b2_c = np.ascontiguousarray(np.stack(
            [bh[oh * O_LOC:(oh + 1) * O_LOC],
             bl[oh * O_LOC:(oh + 1) * O_LOC]]))  # [2, O_LOC]
        in_maps.append({"u0": u0_c, "C": C_c, "b2": b2_c})

    nc = _get_nc()
    return nc, in_maps


def kernel(x: np.ndarray, breakpoints: np.ndarray, values: np.ndarray,
           **_extra) -> np.ndarray:
    nc, in_maps = prepare(x, breakpoints, values)
    res = run_bass_kernel_spmd(nc, in_maps, list(range(N_CORES)))

    outf = np.empty((B, O), np.float32)
    for c in range(N_CORES):
        bq, oh = c % B_SPLIT, c // B_SPLIT
        outf[bq * B_LOC:(bq + 1) * B_LOC, oh * O_LOC:(oh + 1) * O_LOC] = \
            res.results[c]["out"].T.astype(np.float32)
    return outf


if __name__ == "__main__":
    rng = np.random.default_rng(0)
    x = rng.uniform(-1, 1, (B, I)).astype(np.float32)
    bp = np.tile(np.linspace(-1, 1, S + 1, dtype=np.float32), (O, I, 1))
    v = (rng.standard_normal((O, I, S + 1)) * 0.1).astype(np.float32)
    out = kernel(x, bp, v)
    print("kernel ran, out:", out.shape, out.dtype, float(out.std()))


# revision 9
# speedup vs baseline: 1.0773x; 1.0691x over previous
"""Trainium2 Bass kernel for nn_LinearKAN (histogram_binning).

Math
----
reference computes, per (batch b, out o):

    out[b,o] = sum_i  PL_interp(x[b,i]; bp[o,i,:], val[o,i,:])

where bp is the SAME sorted uniform grid for every (o,i) (tiled
linspace).  With u = (x - bp0)/h in [0, S), the piecewise-linear
interpolant has an exact *clamp basis* expansion

    f(u) = val_0 + sum_{s=0..S-1} M_s * clamp(u - s, 0, 1)
    M_s  = val_{s+1} - val_s              (segment slopes)

so the layer is a bias plus S dense matmuls contracting over (s, i).

Device mapping (v3):
  - UNSHIFTED clamp basis: gt_s = min(max(u, s), s+1).  For u (already
    fp16) in (s, s+1) the clamp is a passthrough, and the integer
    saturations are exact in fp16, so the unshifted basis adds NO
    rounding over the shifted one -- PROVIDED the host bias fold is
    computed against the fp16-QUANTIZED device weights (the old fold
    against exact f64 slopes is what made large shifts lossy).
  - u-substitution: sum_s clamp(u-s,0,1) = u identically on [0,S), so
    the s=0 basis function is replaced by u0 itself as a matmul rhs
    (weights D_u = fp16(M_0); other weights become M_s - M_0).  One
    fewer DVE op, no extra tiles.
  - top segment s=19 on the ACT engine as relu(u-19) (u < 20 strictly).
  - 18 interior clamps on DVE, one dual-ALU tensor_scalar each.
  - bias is seeded into PSUM by ONE K=2 matmul (rows: bias_hi,
    bias_lo*2048; rhs rows: ones, ones*2^-11) during the PE warmup
    window; tail is a split PSUM->SBUF copy (DVE half + ACT half) +
    one DMA out (fp16).
  - shard: batch quarters (B_loc=256) x out-feature halves (O_loc=128)
    over 8 cores; no cross-device reduction.

Scheduling notes (from trace iteration):
  - exec_time is measured from the FIRST kernel instruction to the
    absolute end of the NEFF teardown (the ~250-instruction per-engine
    semaphore-reset storm, ~8-9us, is framework-fixed and fully
    counted).  Only the active window [first memset -> out-DMA done]
    is controllable.
  - the 6.5us framework preamble before the first kernel instruction
    is NOT counted.
  - bulk DMA: u0 first on the Sync HWDGE ring (it gates everything),
    the first C chunk rides the ACT ring in parallel, remaining C
    chunks follow u0 on Sync.  Rows >= 2.5KB, movers are
    descriptor-rate-bound (~116ns/packet/engine) below ~2KB rows.
  - PE warmup matmuls (memset scratch) keep full-array duty from
    ~7.0us so the HAM clock boost (observed onset ~13.5us, granted in
    ~6.8us quanta) arrives as early as possible; warmups are N=256 so
    the PE frees within ~1 op of data arrival.  Thin K=1 warmups do
    NOT earn the boost (measured in a previous session).
  - never put tensor_scalar on Pool/gpsimd (software loop, ~7.5us/op).
"""

import os
import numpy as np

import concourse.bass as bass
import concourse.mybir as mybir
import concourse.tile as tile
from concourse import bacc
from concourse.bass_utils import run_bass_kernel_spmd

# Problem shape (hardcoded per the task contract).
B, O, I, S = 1024, 256, 256, 20
N_CORES = 8
B_SPLIT, O_SPLIT = 4, 2
B_LOC, O_LOC = B // B_SPLIT, O // O_SPLIT  # 256, 128
KT = 2 * S          # 40 K-tiles of 128 over the (s, i) contraction
F32 = mybir.dt.float32
F16 = mybir.dt.float16
FW = 2 * B_LOC      # free width of u/g tiles: both i-halves side by side

# Bias is seeded hi/lo: row0 = fp16(bias), row1 = fp16(bias - row0).
# The residual is <= 0.125 (half an fp16 ulp at |bias|~300), comfortably
# normal in fp16, so both rows multiply a single all-ones rhs.


def _envtuple(name, default):
    v = os.environ.get(name)
    if not v:
        return default
    return tuple(int(t) for t in v.split(",") if t != "")


# --- tunables (env-overridable for perf iteration) ---
N_WARMUP_MM = int(os.environ.get("KAN_WARMUP", "11"))  # PE clock-warmup mms
WARM_N = int(os.environ.get("KAN_WARM_N", "256"))      # warmup rhs width
ACT_KT = int(os.environ.get("KAN_ACT_KT", "9"))        # TRAILING C kt, ACT ring
CHUNK_KT = _envtuple("KAN_CHUNKS", (2, 3, 4, 6, 8, 8))  # C kt chunks, Sync ring
OUT_SPLIT = int(os.environ.get("KAN_OUT_SPLIT", "1"))  # out DMA on both rings


def _strip_init_boilerplate(nc) -> None:
    """Drop the Bass-init const-AP memsets + all-engine barrier (~1.5us of
    preamble).  All activation biases here are explicit APs or float biases
    on Copy, so the const-AP memsets and their barrier are dead weight."""
    blk = nc.m.functions[0].blocks[0]
    drop = (mybir.InstMemset, mybir.InstDrain, mybir.InstEventSemaphore)
    keep = [i for i in blk.instructions if not isinstance(i, drop)]
    del blk.instructions[:]
    for i in keep:
        blk.instructions.append(i)
    nc.const_aps.aps.clear()


def _build_nc() -> bass.Bass:
    """Build the (SPMD-identical) single-core Bass graph."""
    assert ACT_KT + sum(CHUNK_KT) == KT, (ACT_KT, CHUNK_KT)
    nc = bacc.Bacc("TRN2", target_bir_lowering=False, debug=False)
    _strip_init_boilerplate(nc)

    u0d = nc.declare_dram_parameter("u0", [128, FW], F16, isOutput=False)
    Cd = nc.declare_dram_parameter("C", [128, KT * 128], F16, isOutput=False)
    b2d = nc.declare_dram_parameter("b2", [2, O_LOC], F16, isOutput=False)
    out = nc.declare_dram_parameter("out", [O_LOC, B_LOC], F16, isOutput=True)

    with tile.TileContext(nc) as tc:
        with (
            tc.tile_pool(name="u", bufs=1) as upool,
            tc.tile_pool(name="g", bufs=S) as gpool,
            tc.tile_pool(name="c", bufs=1 + len(CHUNK_KT)) as cpool,
            tc.tile_pool(name="w", bufs=4) as wpool,
            tc.tile_pool(name="o", bufs=1) as opool,
            tc.tile_pool(name="ps", bufs=2, space="PSUM") as pspool,
        ):
            # --- Pool-engine memsets (no DMA dep).  Warmup operand FIRST:
            # it gates the PE duty ramp that opens the clock boost.
            if N_WARMUP_MM:
                wa = wpool.tile([128, max(128, WARM_N)], F16, tag="warm_a")
                nc.gpsimd.memset(wa[:], 0.0)
            ones2 = wpool.tile([2, B_LOC], F16, tag="ones2")
            nc.gpsimd.memset(ones2[:], 1.0)
            actb = wpool.tile([128, 1], F32, tag="actb")
            nc.gpsimd.memset(actb[:], -float(S - 1))

            # --- PE HAM warmup: dummy matmuls on memset scratch keep
            # full-array duty up while waiting for data.
            if N_WARMUP_MM:
                ps_warm = pspool.tile([128, WARM_N], F32, tag="pw")
                for _ in range(N_WARMUP_MM):
                    nc.tensor.matmul(ps_warm[:], wa[:, 0:128], wa[:, 0:WARM_N],
                                     start=True, stop=True)

            # --- DMA in.  Sync ring: u0 FIRST (it gates everything), then
            # C chunks in consumption order with PROGRESSIVE sizes so the
            # first k-tiles land just ahead of the 1.2GHz-paced stream.
            # ACT ring: tiny bias rows, then the TRAILING C chunk (the ACT
            # ring ramps its movers slowly, so give it only late-needed
            # data) -- this offloads ~9 kt from the Sync ring's tail.
            b2 = wpool.tile([2, O_LOC], F16, tag="b2")
            nc.scalar.dma_start(b2[:], b2d[:])
            u0 = upool.tile([128, FW], F16, tag="u0")
            nc.sync.dma_start(u0[:], u0d[:])
            ckt = {}
            kt0 = 0
            for ci, nkt in enumerate(CHUNK_KT):
                t = cpool.tile([128, nkt * 128], F16, tag=f"c{ci}")
                nc.sync.dma_start(t[:], Cd[:, kt0 * 128:(kt0 + nkt) * 128])
                for k in range(nkt):
                    ckt[kt0 + k] = t[:, k * 128:(k + 1) * 128]
                kt0 += nkt
            if ACT_KT:
                tA = cpool.tile([128, ACT_KT * 128], F16, tag="cA")
                nc.scalar.dma_start(tA[:], Cd[:, kt0 * 128:(kt0 + ACT_KT) * 128])
                for k in range(ACT_KT):
                    ckt[kt0 + k] = tA[:, k * 128:(k + 1) * 128]
                kt0 += ACT_KT
            assert kt0 == KT

            # --- PSUM bias seed: one K=2 matmul, ps = bias_hi + bias_lo.
            ps = pspool.tile([O_LOC, B_LOC], F32, tag="ps")
            nc.tensor.matmul(ps[:], b2[:], ones2[:],
                             start=True, stop=False, skip_group_check=True)

            # --- basis tiles: s=0 is u0 itself (u-substitution); interior
            # s=1..18 are single dual-ALU clamps on DVE; s=19 is relu on ACT.
            g = [None] * S
            g[0] = u0
            for s in range(1, S - 1):
                gs = gpool.tile([128, FW], F16, tag="g")
                nc.vector.tensor_scalar(
                    gs[:], u0[:], float(s), float(s) + 1.0,
                    mybir.AluOpType.max, mybir.AluOpType.min)
                g[s] = gs
            gt = gpool.tile([128, FW], F16, tag="g")
            nc.scalar.activation(
                gt[:], u0[:], mybir.ActivationFunctionType.Relu,
                bias=actb[:, 0:1], scale=1.0)
            g[S - 1] = gt

            # --- 40 accumulating matmuls over kt = (s, ih) ---
            for kt in range(KT):
                s, ih = kt // 2, kt % 2
                rhs = g[s][:, ih * B_LOC:(ih + 1) * B_LOC]
                nc.tensor.matmul(ps[:], ckt[kt], rhs,
                                 start=False, stop=(kt == KT - 1),
                                 skip_group_check=True)

            # --- tail: one DVE PSUM -> SBUF copy (DVE reacts to the last
            # matmul's sem ~10x faster than the idle ACT engine), then the
            # out DMA split across BOTH rings in parallel.
            out_sb = opool.tile([O_LOC, B_LOC], F16, tag="osb")
            nc.vector.tensor_scalar(
                out_sb[:], ps[:], 0.0, None, mybir.AluOpType.add)
            if OUT_SPLIT:
                h = B_LOC // 2
                nc.sync.dma_start(out[:, 0:h], out_sb[:, 0:h])
                nc.scalar.dma_start(out[:, h:B_LOC], out_sb[:, h:B_LOC])
            else:
                nc.sync.dma_start(out[:], out_sb[:])
    nc.compile()
    return nc


_NC_CACHE: dict = {}


def _get_nc() -> bass.Bass:
    if "nc" not in _NC_CACHE:
        _NC_CACHE["nc"] = _build_nc()
    return _NC_CACHE["nc"]


def prepare(x: np.ndarray, breakpoints: np.ndarray, values: np.ndarray):
    """Host prep: build the Bass graph (cached) + per-core input maps."""
    x = np.asarray(x)
    values = np.asarray(values)

    # Grid affine params from the (shared) breakpoint row.
    bpr = np.asarray(breakpoints)[0, 0].astype(np.float64)
    h = (bpr[-1] - bpr[0]) / S
    scale = 1.0 / h
    ubias = -float(bpr[0]) / h

    # u in [0, S) computed on host in f64, shipped fp16.
    u = (x.astype(np.float64) * scale + ubias)
    u16 = u.astype(np.float16)

    # Clamp-basis slopes.  Device weights (all fp16):
    #   kt(s=0) slots: D_u = fp16(M_0)          (rhs = u0 itself)
    #   kt(s>=1) slots: D_s = fp16(M_s - D_u)   (rhs = clamp / relu tiles)
    # Bias fold MUST use the quantized device weights: matching at u=0
    # (all clamps = s, relu = 0, u-term = 0) gives
    #   bias_o = sum_i val0 - sum_{s=1..18} s * sum_i D_s[o,i].
    Vf = values.astype(np.float64)          # [O, I, S+1]
    M = (Vf[:, :, 1:] - Vf[:, :, :-1]).transpose(2, 0, 1)  # [S, O, I] f64
    Du = M[0].astype(np.float16)            # [O, I]
    D16 = np.empty((S, O, I), np.float16)
    D16[0] = Du
    Duf = Du.astype(np.float64)
    for s in range(1, S):
        D16[s] = (M[s] - Duf).astype(np.float16)
    svec = np.arange(1, S - 1, dtype=np.float64)  # 1..18
    bias_o = Vf[:, :, 0].sum(axis=1) - np.einsum(
        "s,soi->o", svec, D16[1:S - 1].astype(np.float64))  # [O] f64
    bh = bias_o.astype(np.float16)
    bl = (bias_o - bh.astype(np.float64)).astype(np.float16)

    # Per-core layouts.
    D16_r = D16.reshape(S, O_SPLIT, O_LOC, 2, 128)  # [s, oh, o, ih, j]
    ur = u16.reshape(B_SPLIT, B_LOC, 2, 128)        # [bq, b, ih, j]

    in_maps = []
    for c in range(N_CORES):
        bq, oh = c % B_SPLIT, c // B_SPLIT
        # ur[bq] axes (b, ih, j) -> (j, ih, b) -> [128, FW]
        u0_c = np.ascontiguousarray(
            ur[bq].transpose(2, 1, 0)).reshape(128, FW)
        # [s, o, ih, j] -> (j, s, ih, o): columns kt*128 + o, kt = 2s+ih
        C_c = np.ascontiguousarray(
            D16_r[:, oh].transpose(3, 0, 2, 1)).reshape(128, KT * 128)
        b2_c = np.ascontiguousarray(np.stack(
            [bh[oh * O_LOC:(oh + 1) * O_LOC],
             bl[oh * O_LOC:(oh + 1) * O_LOC]]))  # [2, O_LOC]
        in_maps.append({"u0": u0_c, "C": C_c, "b2": b2_c})

    nc = _get_nc()
    return nc, in_maps


def kernel(x: np.ndarray, breakpoints: np.ndarray, values: np.ndarray,
           **_extra) -> np.ndarray:
    nc, in_maps = prepare(x, breakpoints, values)
    res = run_bass_kernel_spmd(nc, in_maps, list(range(N_CORES)))

    outf = np.empty((B, O), np.float32)
    for c in range(N_CORES):
        bq, oh = c % B_SPLIT, c // B_SPLIT
        outf[bq * B_LOC:(bq + 1) * B_LOC, oh * O_LOC:(oh + 1) * O_LOC] = \
            res.results[c]["out"].T.astype(np.float32)
    return outf


if __name__ == "__main__":
    rng = np.random.default_rng(0)
    x = rng.uniform(-1, 1, (B, I)).astype(np.float32)
    bp = np.tile(np.linspace(-1, 1, S + 1, dtype=np.float32), (O, I, 1))
    v = (rng.standard_normal((O, I, S + 1)) * 0.1).astype(np.float32)
    out = kernel(x, bp, v)
    print("kernel ran, out:", out.shape, out.dtype, float(out.std()))


# revision 17
# speedup vs baseline: 1.1180x; 1.0378x over previous
"""Trainium2 Bass kernel for nn_LinearKAN (histogram_binning).

Math
----
reference computes, per (batch b, out o):

    out[b,o] = sum_i  PL_interp(x[b,i]; bp[o,i,:], val[o,i,:])

where bp is the SAME sorted uniform grid for every (o,i) (tiled
linspace).  With u = (x - bp0)/h in [0, S), the piecewise-linear
interpolant has an exact *clamp basis* expansion

    f(u) = val_0 + sum_{s=0..S-1} M_s * clamp(u - s, 0, 1)
    M_s  = val_{s+1} - val_s              (segment slopes)

so the layer is a bias plus S dense matmuls contracting over (s, i).

Device mapping (v3):
  - UNSHIFTED clamp basis: gt_s = min(max(u, s), s+1).  For u (already
    fp16) in (s, s+1) the clamp is a passthrough, and the integer
    saturations are exact in fp16, so the unshifted basis adds NO
    rounding over the shifted one -- PROVIDED the host bias fold is
    computed against the fp16-QUANTIZED device weights (the old fold
    against exact f64 slopes is what made large shifts lossy).
  - u-substitution: sum_s clamp(u-s,0,1) = u identically on [0,S), so
    the s=0 basis function is replaced by u0 itself as a matmul rhs
    (weights D_u = fp16(M_0); other weights become M_s - M_0).  One
    fewer DVE op, no extra tiles.
  - top segment s=19 on the ACT engine as relu(u-19) (u < 20 strictly).
  - 18 interior clamps on DVE, one dual-ALU tensor_scalar each.
  - bias is seeded into PSUM by ONE K=2 matmul (rows: bias_hi,
    bias_lo*2048; rhs rows: ones, ones*2^-11) during the PE warmup
    window; tail is a split PSUM->SBUF copy (DVE half + ACT half) +
    one DMA out (fp16).
  - shard: batch quarters (B_loc=256) x out-feature halves (O_loc=128)
    over 8 cores; no cross-device reduction.

Scheduling notes (from trace iteration):
  - exec_time is measured from the FIRST kernel instruction to the
    absolute end of the NEFF teardown (the ~250-instruction per-engine
    semaphore-reset storm, ~8-9us, is framework-fixed and fully
    counted).  Only the active window [first memset -> out-DMA done]
    is controllable.
  - the 6.5us framework preamble before the first kernel instruction
    is NOT counted.
  - bulk DMA: u0 first on the Sync HWDGE ring (it gates everything),
    the first C chunk rides the ACT ring in parallel, remaining C
    chunks follow u0 on Sync.  Rows >= 2.5KB, movers are
    descriptor-rate-bound (~116ns/packet/engine) below ~2KB rows.
  - PE warmup matmuls (memset scratch) keep full-array duty from
    ~7.0us so the HAM clock boost (observed onset ~13.5us, granted in
    ~6.8us quanta) arrives as early as possible; warmups are N=256 so
    the PE frees within ~1 op of data arrival.  Thin K=1 warmups do
    NOT earn the boost (measured in a previous session).
  - never put tensor_scalar on Pool/gpsimd (software loop, ~7.5us/op).
"""

import os
import numpy as np

import concourse.bass as bass
import concourse.mybir as mybir
import concourse.tile as tile
from concourse import bacc
from concourse.bass_utils import run_bass_kernel_spmd

# Problem shape (hardcoded per the task contract).
B, O, I, S = 1024, 256, 256, 20
N_CORES = 8
B_SPLIT, O_SPLIT = 4, 2
B_LOC, O_LOC = B // B_SPLIT, O // O_SPLIT  # 256, 128
KT = 2 * S          # 40 K-tiles of 128 over the (s, i) contraction
F32 = mybir.dt.float32
F16 = mybir.dt.float16
FW = 2 * B_LOC      # free width of u/g tiles: both i-halves side by side

# Bias is seeded hi/lo: row0 = fp16(bias), row1 = fp16(bias - row0).
# The residual is <= 0.125 (half an fp16 ulp at |bias|~300), comfortably
# normal in fp16, so both rows multiply a single all-ones rhs.


def _envtuple(name, default):
    v = os.environ.get(name)
    if not v:
        return default
    return tuple(int(t) for t in v.split(",") if t != "")


# --- tunables (env-overridable for perf iteration) ---
N_WARMUP_MM = int(os.environ.get("KAN_WARMUP", "16"))  # PE clock-warmup mms
WARM_N = int(os.environ.get("KAN_WARM_N", "256"))      # warmup rhs width
CHUNK_KT = _envtuple("KAN_CHUNKS", (10, 14, 16))       # C kt chunks, Sync ring
RING_WAKE = int(os.environ.get("KAN_RING_WAKE", "1"))  # dummy DMA wakes Sync ring


def _strip_init_boilerplate(nc) -> None:
    """Drop the Bass-init const-AP memsets + all-engine barrier (~1.5us of
    preamble).  All activation biases here are explicit APs or float biases
    on Copy, so the const-AP memsets and their barrier are dead weight."""
    blk = nc.m.functions[0].blocks[0]
    drop = (mybir.InstMemset, mybir.InstDrain, mybir.InstEventSemaphore)
    keep = [i for i in blk.instructions if not isinstance(i, drop)]
    del blk.instructions[:]
    for i in keep:
        blk.instructions.append(i)
    nc.const_aps.aps.clear()


def _build_nc() -> bass.Bass:
    """Build the (SPMD-identical) single-core Bass graph."""
    assert sum(CHUNK_KT) == KT, CHUNK_KT
    nc = bacc.Bacc("TRN2", target_bir_lowering=False, debug=False)
    _strip_init_boilerplate(nc)

    u0d = nc.declare_dram_parameter("u0", [128, FW], F16, isOutput=False)
    Cd = nc.declare_dram_parameter("C", [128, KT * 128], F16, isOutput=False)
    b2d = nc.declare_dram_parameter("b2", [2, O_LOC], F16, isOutput=False)
    out = nc.declare_dram_parameter("out", [O_LOC, B_LOC], F16, isOutput=True)

    with tile.TileContext(nc) as tc:
        with (
            tc.tile_pool(name="u", bufs=1) as upool,
            tc.tile_pool(name="g", bufs=S) as gpool,
            tc.tile_pool(name="c", bufs=1 + len(CHUNK_KT)) as cpool,
            tc.tile_pool(name="w", bufs=4) as wpool,
            tc.tile_pool(name="o", bufs=1) as opool,
            tc.tile_pool(name="ps", bufs=2, space="PSUM") as pspool,
        ):
            # --- Pool-engine memsets (no DMA dep).  Warmup operand FIRST:
            # it gates the PE duty ramp that opens the clock boost.
            if N_WARMUP_MM:
                wa = wpool.tile([128, max(128, WARM_N)], F16, tag="warm_a")
                nc.gpsimd.memset(wa[:], 0.0)
            ones2 = wpool.tile([2, B_LOC], F16, tag="ones2")
            nc.gpsimd.memset(ones2[:], 1.0)
            actb = wpool.tile([128, 1], F32, tag="actb")
            nc.gpsimd.memset(actb[:], -float(S - 1))

            # --- PE HAM warmup: dummy matmuls on memset scratch keep
            # full-array duty up while waiting for data.
            if N_WARMUP_MM:
                ps_warm = pspool.tile([128, WARM_N], F32, tag="pw")
                for _ in range(N_WARMUP_MM):
                    nc.tensor.matmul(ps_warm[:], wa[:, 0:128], wa[:, 0:WARM_N],
                                     start=True, stop=True)

            # --- DMA in.  DMA packets are per-partition-row: every DMA
            # costs 8 packets/engine (~130-230ns each) REGARDLESS of row
            # size, so few, large C chunks beat many small ones.  Sync
            # ring: a dummy 32B wake DMA (hides the ~1.5us ring-start
            # latency behind the u0 descriptor build), u0, then 3 C
            # chunks.  ACT ring: just the tiny bias rows.
            if RING_WAKE:
                wake = wpool.tile([1, 16], F16, tag="wake")
                nc.sync.dma_start(wake[:], b2d[0:1, 0:16])
            b2 = wpool.tile([2, O_LOC], F16, tag="b2")
            nc.scalar.dma_start(b2[:], b2d[:])
            u0 = upool.tile([128, FW], F16, tag="u0")
            nc.sync.dma_start(u0[:], u0d[:])
            ckt = {}
            kt0 = 0
            for ci, nkt in enumerate(CHUNK_KT):
                t = cpool.tile([128, nkt * 128], F16, tag=f"c{ci}")
                nc.sync.dma_start(t[:], Cd[:, kt0 * 128:(kt0 + nkt) * 128])
                for k in range(nkt):
                    ckt[kt0 + k] = t[:, k * 128:(k + 1) * 128]
                kt0 += nkt
            assert kt0 == KT

            # --- PSUM bias seed: one K=2 matmul, ps = bias_hi + bias_lo.
            ps = pspool.tile([O_LOC, B_LOC], F32, tag="ps")
            nc.tensor.matmul(ps[:], b2[:], ones2[:],
                             start=True, stop=False, skip_group_check=True)

            # --- basis tiles: s=0 is u0 itself (u-substitution); interior
            # s=1..18 are single dual-ALU clamps on DVE; s=19 is relu on ACT.
            g = [None] * S
            g[0] = u0
            for s in range(1, S - 1):
                gs = gpool.tile([128, FW], F16, tag="g")
                nc.vector.tensor_scalar(
                    gs[:], u0[:], float(s), float(s) + 1.0,
                    mybir.AluOpType.max, mybir.AluOpType.min)
                g[s] = gs
            gt = gpool.tile([128, FW], F16, tag="g")
            nc.scalar.activation(
                gt[:], u0[:], mybir.ActivationFunctionType.Relu,
                bias=actb[:, 0:1], scale=1.0)
            g[S - 1] = gt

            # --- 40 accumulating matmuls over kt = (s, ih) ---
            for kt in range(KT):
                s, ih = kt // 2, kt % 2
                rhs = g[s][:, ih * B_LOC:(ih + 1) * B_LOC]
                nc.tensor.matmul(ps[:], ckt[kt], rhs,
                                 start=False, stop=(kt == KT - 1),
                                 skip_group_check=True)

            # --- tail: DVE PSUM -> SBUF copy (DVE reacts to the last
            # matmul's sem fast; the idle ACT engine adds ~0.4us of wake
            # lag), then the out DMA split across both rings in parallel.
            out_sb = opool.tile([O_LOC, B_LOC], F16, tag="osb")
            h = B_LOC // 2
            nc.vector.tensor_scalar(
                out_sb[:, 0:h], ps[:, 0:h], 0.0, None, mybir.AluOpType.add)
            nc.sync.dma_start(out[:, 0:h], out_sb[:, 0:h])
            nc.vector.tensor_scalar(
                out_sb[:, h:B_LOC], ps[:, h:B_LOC], 0.0, None,
                mybir.AluOpType.add)
            nc.scalar.dma_start(out[:, h:B_LOC], out_sb[:, h:B_LOC])
    nc.compile()
    return nc


_NC_CACHE: dict = {}


def _get_nc() -> bass.Bass:
    if "nc" not in _NC_CACHE:
        _NC_CACHE["nc"] = _build_nc()
    return _NC_CACHE["nc"]


def prepare(x: np.ndarray, breakpoints: np.ndarray, values: np.ndarray):
    """Host prep: build the Bass graph (cached) + per-core input maps."""
    x = np.asarray(x)
    values = np.asarray(values)

    # Grid affine params from the (shared) breakpoint row.
    bpr = np.asarray(breakpoints)[0, 0].astype(np.float64)
    h = (bpr[-1] - bpr[0]) / S
    scale = 1.0 / h
    ubias = -float(bpr[0]) / h

    # u in [0, S) computed on host in f64, shipped fp16.
    u = (x.astype(np.float64) * scale + ubias)
    u16 = u.astype(np.float16)

    # Clamp-basis slopes.  Device weights (all fp16):
    #   kt(s=0) slots: D_u = fp16(M_0)          (rhs = u0 itself)
    #   kt(s>=1) slots: D_s = fp16(M_s - D_u)   (rhs = clamp / relu tiles)
    # Bias fold MUST use the quantized device weights: matching at u=0
    # (all clamps = s, relu = 0, u-term = 0) gives
    #   bias_o = sum_i val0 - sum_{s=1..18} s * sum_i D_s[o,i].
    Vf = values.astype(np.float64)          # [O, I, S+1]
    M = (Vf[:, :, 1:] - Vf[:, :, :-1]).transpose(2, 0, 1)  # [S, O, I] f64
    Du = M[0].astype(np.float16)            # [O, I]
    D16 = np.empty((S, O, I), np.float16)
    D16[0] = Du
    Duf = Du.astype(np.float64)
    for s in range(1, S):
        D16[s] = (M[s] - Duf).astype(np.float16)
    svec = np.arange(1, S - 1, dtype=np.float64)  # 1..18
    bias_o = Vf[:, :, 0].sum(axis=1) - np.einsum(
        "s,soi->o", svec, D16[1:S - 1].astype(np.float64))  # [O] f64
    bh = bias_o.astype(np.float16)
    bl = (bias_o - bh.astype(np.float64)).astype(np.float16)

    # Per-core layouts.
    D16_r = D16.reshape(S, O_SPLIT, O_LOC, 2, 128)  # [s, oh, o, ih, j]
    ur = u16.reshape(B_SPLIT, B_LOC, 2, 128)        # [bq, b, ih, j]

    in_maps = []
    for c in range(N_CORES):
        bq, oh = c % B_SPLIT, c // B_SPLIT
        # ur[bq] axes (b, ih, j) -> (j, ih, b) -> [128, FW]
        u0_c = np.ascontiguousarray(
            ur[bq].transpose(2, 1, 0)).reshape(128, FW)
        # [s, o, ih, j] -> (j, s, ih, o): columns kt*128 + o, kt = 2s+ih
        C_c = np.ascontiguousarray(
            D16_r[:, oh].transpose(3, 0, 2, 1)).reshape(128, KT * 128)
        b2_c = np.ascontiguousarray(np.stack(
            [bh[oh * O_LOC:(oh + 1) * O_LOC],
             bl[oh * O_LOC:(oh + 1) * O_LOC]]))  # [2, O_LOC]
        in_maps.append({"u0": u0_c, "C": C_c, "b2": b2_c})

    nc = _get_nc()
    return nc, in_maps


def kernel(x: np.ndarray, breakpoints: np.ndarray, values: np.ndarray,
           **_extra) -> np.ndarray:
    nc, in_maps = prepare(x, breakpoints, values)
    res = run_bass_kernel_spmd(nc, in_maps, list(range(N_CORES)))

    outf = np.empty((B, O), np.float32)
    for c in range(N_CORES):
        bq, oh = c % B_SPLIT, c // B_SPLIT
        outf[bq * B_LOC:(bq + 1) * B_LOC, oh * O_LOC:(oh + 1) * O_LOC] = \
            res.results[c]["out"].T.astype(np.float32)
    return outf


if __name__ == "__main__":
    rng = np.random.default_rng(0)
    x = rng.uniform(-1, 1, (B, I)).astype(np.float32)
    bp = np.tile(np.linspace(-1, 1, S + 1, dtype=np.float32), (O, I, 1))
    v = (rng.standard_normal((O, I, S + 1)) * 0.1).astype(np.float32)
    out = kernel(x, bp, v)
    print("kernel ran, out:", out.shape, out.dtype, float(out.std()))


# revision 19
# speedup vs baseline: 1.2087x; 1.0811x over previous
"""Trainium2 Bass kernel for nn_LinearKAN (histogram_binning).

Math
----
reference computes, per (batch b, out o):

    out[b,o] = sum_i  PL_interp(x[b,i]; bp[o,i,:], val[o,i,:])

where bp is the SAME sorted uniform grid for every (o,i) (tiled
linspace).  With u = (x - bp0)/h in [0, S), the piecewise-linear
interpolant has an exact *clamp basis* expansion

    f(u) = val_0 + sum_{s=0..S-1} M_s * clamp(u - s, 0, 1)
    M_s  = val_{s+1} - val_s              (segment slopes)

so the layer is a bias plus S dense matmuls contracting over (s, i).

Device mapping (v3):
  - UNSHIFTED clamp basis: gt_s = min(max(u, s), s+1).  For u (already
    fp16) in (s, s+1) the clamp is a passthrough, and the integer
    saturations are exact in fp16, so the unshifted basis adds NO
    rounding over the shifted one -- PROVIDED the host bias fold is
    computed against the fp16-QUANTIZED device weights (the old fold
    against exact f64 slopes is what made large shifts lossy).
  - u-substitution: sum_s clamp(u-s,0,1) = u identically on [0,S), so
    the s=0 basis function is replaced by u0 itself as a matmul rhs
    (weights D_u = fp16(M_0); other weights become M_s - M_0).  One
    fewer DVE op, no extra tiles.
  - top segment s=19 on the ACT engine as relu(u-19) (u < 20 strictly).
  - 18 interior clamps on DVE, one dual-ALU tensor_scalar each.
  - bias is seeded into PSUM by ONE K=2 matmul (rows: bias_hi,
    bias_lo*2048; rhs rows: ones, ones*2^-11) during the PE warmup
    window; tail is a split PSUM->SBUF copy (DVE half + ACT half) +
    one DMA out (fp16).
  - shard: batch quarters (B_loc=256) x out-feature halves (O_loc=128)
    over 8 cores; no cross-device reduction.

Scheduling notes (from trace iteration):
  - exec_time is measured from the FIRST kernel instruction to the
    absolute end of the NEFF teardown (the ~250-instruction per-engine
    semaphore-reset storm, ~8-9us, is framework-fixed and fully
    counted).  Only the active window [first memset -> out-DMA done]
    is controllable.
  - the 6.5us framework preamble before the first kernel instruction
    is NOT counted.
  - bulk DMA: u0 first on the Sync HWDGE ring (it gates everything),
    the first C chunk rides the ACT ring in parallel, remaining C
    chunks follow u0 on Sync.  Rows >= 2.5KB, movers are
    descriptor-rate-bound (~116ns/packet/engine) below ~2KB rows.
  - PE warmup matmuls (memset scratch) keep full-array duty from
    ~7.0us so the HAM clock boost (observed onset ~13.5us, granted in
    ~6.8us quanta) arrives as early as possible; warmups are N=256 so
    the PE frees within ~1 op of data arrival.  Thin K=1 warmups do
    NOT earn the boost (measured in a previous session).
  - never put tensor_scalar on Pool/gpsimd (software loop, ~7.5us/op).
"""

import os
import numpy as np

import concourse.bass as bass
import concourse.mybir as mybir
import concourse.tile as tile
from concourse import bacc
from concourse.bass_utils import run_bass_kernel_spmd

# Problem shape (hardcoded per the task contract).
B, O, I, S = 1024, 256, 256, 20
N_CORES = 8
B_SPLIT, O_SPLIT = 4, 2
B_LOC, O_LOC = B // B_SPLIT, O // O_SPLIT  # 256, 128
KT = 2 * S          # 40 K-tiles of 128 over the (s, i) contraction
F32 = mybir.dt.float32
F16 = mybir.dt.float16
FW = 2 * B_LOC      # free width of u/g tiles: both i-halves side by side

# Bias is seeded hi/lo: row0 = fp16(bias), row1 = fp16(bias - row0).
# The residual is <= 0.125 (half an fp16 ulp at |bias|~300), comfortably
# normal in fp16, so both rows multiply a single all-ones rhs.


def _envtuple(name, default):
    v = os.environ.get(name)
    if not v:
        return default
    return tuple(int(t) for t in v.split(",") if t != "")


# --- tunables (env-overridable for perf iteration) ---
N_WARMUP_MM = int(os.environ.get("KAN_WARMUP", "18"))  # PE clock-warmup mms
WARM_N = int(os.environ.get("KAN_WARM_N", "256"))      # warmup rhs width
CHUNK_KT = _envtuple("KAN_CHUNKS", (10, 14, 16))       # C kt chunks, Sync ring
U0_SPLIT = int(os.environ.get("KAN_U0_SPLIT", "1"))    # u0 halves on both rings


def _strip_init_boilerplate(nc) -> None:
    """Drop the Bass-init const-AP memsets + all-engine barrier (~1.5us of
    preamble).  All activation biases here are explicit APs or float biases
    on Copy, so the const-AP memsets and their barrier are dead weight."""
    blk = nc.m.functions[0].blocks[0]
    drop = (mybir.InstMemset, mybir.InstDrain, mybir.InstEventSemaphore)
    keep = [i for i in blk.instructions if not isinstance(i, drop)]
    del blk.instructions[:]
    for i in keep:
        blk.instructions.append(i)
    nc.const_aps.aps.clear()


def _build_nc() -> bass.Bass:
    """Build the (SPMD-identical) single-core Bass graph."""
    assert sum(CHUNK_KT) == KT, CHUNK_KT
    nc = bacc.Bacc("TRN2", target_bir_lowering=False, debug=False)
    _strip_init_boilerplate(nc)

    u0d = nc.declare_dram_parameter("u0", [128, FW], F16, isOutput=False)
    Cd = nc.declare_dram_parameter("C", [128, KT * 128], F16, isOutput=False)
    b2d = nc.declare_dram_parameter("b2", [2, O_LOC], F16, isOutput=False)
    out = nc.declare_dram_parameter("out", [O_LOC, B_LOC], F16, isOutput=True)

    with tile.TileContext(nc) as tc:
        with (
            tc.tile_pool(name="u", bufs=1) as upool,
            tc.tile_pool(name="g", bufs=S) as gpool,
            tc.tile_pool(name="c", bufs=1 + len(CHUNK_KT)) as cpool,
            tc.tile_pool(name="w", bufs=4) as wpool,
            tc.tile_pool(name="o", bufs=1) as opool,
            tc.tile_pool(name="ps", bufs=2, space="PSUM") as pspool,
        ):
            # --- Pool-engine memsets (no DMA dep).  Warmup operand FIRST:
            # it gates the PE duty ramp that opens the clock boost.
            if N_WARMUP_MM:
                wa = wpool.tile([128, max(128, WARM_N)], F16, tag="warm_a")
                nc.gpsimd.memset(wa[:], 0.0)
            ones2 = wpool.tile([2, B_LOC], F16, tag="ones2")
            nc.gpsimd.memset(ones2[:], 1.0)
            actb = wpool.tile([128, 1], F32, tag="actb")
            nc.gpsimd.memset(actb[:], -float(S - 1))

            # --- PE HAM warmup: dummy matmuls on memset scratch keep
            # full-array duty up while waiting for data.
            if N_WARMUP_MM:
                ps_warm = pspool.tile([128, WARM_N], F32, tag="pw")
                for _ in range(N_WARMUP_MM):
                    nc.tensor.matmul(ps_warm[:], wa[:, 0:128], wa[:, 0:WARM_N],
                                     start=True, stop=True)

            # --- DMA in.  DMA packets are per-partition-row: every DMA
            # costs 8 packets/engine (~150-230ns each) REGARDLESS of row
            # size, so few, large C chunks beat many small ones, and the
            # u0 halves ride BOTH rings in parallel (4 pkts/engine each).
            # The ring only starts ~0.8us after the issuing instruction
            # completes, so issue order = priority order.
            u0 = upool.tile([128, FW], F16, tag="u0")
            if U0_SPLIT:
                nc.sync.dma_start(u0[:, 0:B_LOC], u0d[:, 0:B_LOC])
                nc.scalar.dma_start(u0[:, B_LOC:FW], u0d[:, B_LOC:FW])
            else:
                nc.sync.dma_start(u0[:], u0d[:])
            b2 = wpool.tile([2, O_LOC], F16, tag="b2")
            nc.scalar.dma_start(b2[:], b2d[:])
            ckt = {}
            kt0 = 0
            for ci, nkt in enumerate(CHUNK_KT):
                t = cpool.tile([128, nkt * 128], F16, tag=f"c{ci}")
                nc.sync.dma_start(t[:], Cd[:, kt0 * 128:(kt0 + nkt) * 128])
                for k in range(nkt):
                    ckt[kt0 + k] = t[:, k * 128:(k + 1) * 128]
                kt0 += nkt
            assert kt0 == KT

            # --- PSUM bias seed: one K=2 matmul, ps = bias_hi + bias_lo.
            ps = pspool.tile([O_LOC, B_LOC], F32, tag="ps")
            nc.tensor.matmul(ps[:], b2[:], ones2[:],
                             start=True, stop=False, skip_group_check=True)

            # --- basis tiles: s=0 is u0 itself (u-substitution); interior
            # s=1..18 are single dual-ALU clamps on DVE; s=19 is relu on ACT.
            g = [None] * S
            g[0] = u0
            for s in range(1, S - 1):
                gs = gpool.tile([128, FW], F16, tag="g")
                nc.vector.tensor_scalar(
                    gs[:], u0[:], float(s), float(s) + 1.0,
                    mybir.AluOpType.max, mybir.AluOpType.min)
                g[s] = gs
            gt = gpool.tile([128, FW], F16, tag="g")
            nc.scalar.activation(
                gt[:], u0[:], mybir.ActivationFunctionType.Relu,
                bias=actb[:, 0:1], scale=1.0)
            g[S - 1] = gt

            # --- 40 accumulating matmuls over kt = (s, ih) ---
            for kt in range(KT):
                s, ih = kt // 2, kt % 2
                rhs = g[s][:, ih * B_LOC:(ih + 1) * B_LOC]
                nc.tensor.matmul(ps[:], ckt[kt], rhs,
                                 start=False, stop=(kt == KT - 1),
                                 skip_group_check=True)

            # --- tail: DVE PSUM -> SBUF copy (DVE reacts to the last
            # matmul's sem fast; the idle ACT engine adds ~0.4us of wake
            # lag), then the out DMA split across both rings in parallel.
            out_sb = opool.tile([O_LOC, B_LOC], F16, tag="osb")
            h = B_LOC // 2
            nc.vector.tensor_scalar(
                out_sb[:, 0:h], ps[:, 0:h], 0.0, None, mybir.AluOpType.add)
            nc.sync.dma_start(out[:, 0:h], out_sb[:, 0:h])
            nc.vector.tensor_scalar(
                out_sb[:, h:B_LOC], ps[:, h:B_LOC], 0.0, None,
                mybir.AluOpType.add)
            nc.scalar.dma_start(out[:, h:B_LOC], out_sb[:, h:B_LOC])
    nc.compile()
    return nc


_NC_CACHE: dict = {}


def _get_nc() -> bass.Bass:
    if "nc" not in _NC_CACHE:
        _NC_CACHE["nc"] = _build_nc()
    return _NC_CACHE["nc"]


def prepare(x: np.ndarray, breakpoints: np.ndarray, values: np.ndarray):
    """Host prep: build the Bass graph (cached) + per-core input maps."""
    x = np.asarray(x)
    values = np.asarray(values)

    # Grid affine params from the (shared) breakpoint row.
    bpr = np.asarray(breakpoints)[0, 0].astype(np.float64)
    h = (bpr[-1] - bpr[0]) / S
    scale = 1.0 / h
    ubias = -float(bpr[0]) / h

    # u in [0, S) computed on host in f64, shipped fp16.
    u = (x.astype(np.float64) * scale + ubias)
    u16 = u.astype(np.float16)

    # Clamp-basis slopes.  Device weights (all fp16):
    #   kt(s=0) slots: D_u = fp16(M_0)          (rhs = u0 itself)
    #   kt(s>=1) slots: D_s = fp16(M_s - D_u)   (rhs = clamp / relu tiles)
    # Bias fold MUST use the quantized device weights: matching at u=0
    # (all clamps = s, relu = 0, u-term = 0) gives
    #   bias_o = sum_i val0 - sum_{s=1..18} s * sum_i D_s[o,i].
    Vf = values.astype(np.float64)          # [O, I, S+1]
    M = (Vf[:, :, 1:] - Vf[:, :, :-1]).transpose(2, 0, 1)  # [S, O, I] f64
    Du = M[0].astype(np.float16)            # [O, I]
    D16 = np.empty((S, O, I), np.float16)
    D16[0] = Du
    Duf = Du.astype(np.float64)
    for s in range(1, S):
        D16[s] = (M[s] - Duf).astype(np.float16)
    svec = np.arange(1, S - 1, dtype=np.float64)  # 1..18
    bias_o = Vf[:, :, 0].sum(axis=1) - np.einsum(
        "s,soi->o", svec, D16[1:S - 1].astype(np.float64))  # [O] f64
    bh = bias_o.astype(np.float16)
    bl = (bias_o - bh.astype(np.float64)).astype(np.float16)

    # Per-core layouts.
    D16_r = D16.reshape(S, O_SPLIT, O_LOC, 2, 128)  # [s, oh, o, ih, j]
    ur = u16.reshape(B_SPLIT, B_LOC, 2, 128)        # [bq, b, ih, j]

    in_maps = []
    for c in range(N_CORES):
        bq, oh = c % B_SPLIT, c // B_SPLIT
        # ur[bq] axes (b, ih, j) -> (j, ih, b) -> [128, FW]
        u0_c = np.ascontiguousarray(
            ur[bq].transpose(2, 1, 0)).reshape(128, FW)
        # [s, o, ih, j] -> (j, s, ih, o): columns kt*128 + o, kt = 2s+ih
        C_c = np.ascontiguousarray(
            D16_r[:, oh].transpose(3, 0, 2, 1)).reshape(128, KT * 128)
        b2_c = np.ascontiguousarray(np.stack(
            [bh[oh * O_LOC:(oh + 1) * O_LOC],
             bl[oh * O_LOC:(oh + 1) * O_LOC]]))  # [2, O_LOC]
        in_maps.append({"u0": u0_c, "C": C_c, "b2": b2_c})

    nc = _get_nc()
    return nc, in_maps


def kernel(x: np.ndarray, breakpoints: np.ndarray, values: np.ndarray,
           **_extra) -> np.ndarray:
    nc, in_maps = prepare(x, breakpoints, values)
    res = run_bass_kernel_spmd(nc, in_maps, list(range(N_CORES)))

    outf = np.empty((B, O), np.float32)
    for c in range(N_CORES):
        bq, oh = c % B_SPLIT, c // B_SPLIT
        outf[bq * B_LOC:(bq + 1) * B_LOC, oh * O_LOC:(oh + 1) * O_LOC] = \
            res.results[c]["out"].T.astype(np.float32)
    return outf


if __name__ == "__main__":
    rng = np.random.default_rng(0)
    x = rng.uniform(-1, 1, (B, I)).astype(np.float32)
    bp = np.tile(np.linspace(-1, 1, S + 1, dtype=np.float32), (O, I, 1))
    v = (rng.standard_normal((O, I, S + 1)) * 0.1).astype(np.float32)
    out = kernel(x, bp, v)
    print("kernel ran, out:", out.shape, out.dtype, float(out.std()))
